# revision 1
# baseline (speedup 1.0000x reference)
"""Trainium kernel for nn_AttnNetwork: seq2seq LSTM + attention + CE loss.

Strategy (per sharding hint): data-parallel over batch across the 8
NeuronCores. B=64 -> 8 rows per core. LSTM/linear/embedding params are
replicated; each core computes its batch shard's encoder, decoder,
attention and vocab logits (full 1024->32000 projection for its rows),
reducing to per-(b,t) NLL on device. Host combines the tiny [64,32] NLL
into the scalar loss.
"""

import jax
import jax.numpy as jnp
import numpy as np
from functools import partial

H = 512
N_CORES = 8
B, S, T = 64, 32, 33


def _lstm_layer(x, h0, c0, p, reverse=False):
    xs = jnp.swapaxes(x, 0, 1)
    if reverse:
        xs = xs[::-1]
    xw = xs @ p['Wih'].T + p['bih']

    def step(carry, xt):
        h, c = carry
        g = xt + h @ p['Whh'].T + p['bhh']
        i, f, gg, o = jnp.split(g, 4, axis=-1)
        c = jax.nn.sigmoid(f) * c + jax.nn.sigmoid(i) * jnp.tanh(gg)
        h = jax.nn.sigmoid(o) * jnp.tanh(c)
        return (h, c), h

    (hT, cT), hs = jax.lax.scan(step, (h0, c0), xw)
    if reverse:
        hs = hs[::-1]
    return jnp.swapaxes(hs, 0, 1), hT, cT


def _shard_fn(x, y, emb_de, emb_en, enc_params, dec_params,
              W_E, b_E, W_D, b_D, W_A, b_A, W_C, b_C):
    """Full forward for one batch shard; returns per-(b,t) NLL [Bs, T-1]."""
    Bs = x.shape[0]
    zeros = jnp.zeros((Bs, H), jnp.float32)

    ex = emb_de[x]
    ey = emb_en[y]

    inp = ex
    h_fin, c_fin = [], []
    for layer in enc_params:
        hf, hTf, cTf = _lstm_layer(inp, zeros, zeros, layer['fwd'])
        hb, hTb, cTb = _lstm_layer(inp, zeros, zeros, layer['bwd'], reverse=True)
        inp = jnp.concatenate([hf, hb], axis=-1)
        h_fin += [hTf, hTb]
        c_fin += [cTf, cTb]
    enc_h = inp

    dinp = ey[:, :-1]
    for l, layer in enumerate(dec_params):
        dinp, _, _ = _lstm_layer(dinp, h_fin[l], c_fin[l], layer)
    dec_h = dinp

    e_proj = enc_h @ W_E.T + b_E
    d_proj = dec_h @ W_D.T + b_D
    scores = jnp.einsum('bsk,btk->bst', e_proj, d_proj)
    attn = jax.nn.softmax(scores, axis=1)
    context = jnp.einsum('bst,bsk->btk', attn, enc_h)

    t = jnp.tanh(jnp.concatenate([context, dec_h], axis=-1) @ W_A.T + b_A)
    pred = t @ W_C.T + b_C

    labels = y[:, 1:]
    logp = jax.nn.log_softmax(pred, axis=-1)
    nll = -jnp.take_along_axis(logp, labels[..., None], axis=-1)[..., 0]
    return nll


_pmapped = None


def _get_pmapped():
    global _pmapped
    if _pmapped is None:
        _pmapped = jax.pmap(_shard_fn, axis_name='i',
                            in_axes=(0, 0) + (None,) * 12)
    return _pmapped


def kernel(x, y, emb_de, emb_en, enc_params, dec_params,
           W_E, b_E, W_D, b_D, W_A, b_A, W_C, b_C):
    x = np.asarray(x)
    y = np.asarray(y)

    to_f32 = lambda a: np.asarray(a, dtype=np.float32)
    emb_de = to_f32(emb_de)
    emb_en = to_f32(emb_en)
    enc_params = jax.tree_util.tree_map(to_f32, enc_params)
    dec_params = jax.tree_util.tree_map(to_f32, dec_params)
    W_E, b_E = to_f32(W_E), to_f32(b_E)
    W_D, b_D = to_f32(W_D), to_f32(b_D)
    W_A, b_A = to_f32(W_A), to_f32(b_A)
    W_C, b_C = to_f32(W_C), to_f32(b_C)

    nb = x.shape[0]
    bs = nb // N_CORES  # 8 rows per core
    xs = x.reshape(N_CORES, bs, x.shape[1])
    ys = y.reshape(N_CORES, bs, y.shape[1])

    fn = _get_pmapped()
    nll = fn(xs, ys, emb_de, emb_en, enc_params, dec_params,
             W_E, b_E, W_D, b_D, W_A, b_A, W_C, b_C)
    nll = np.asarray(nll).reshape(nb, -1)  # [B, T-1]

    loss = nll.mean(axis=0).sum()
    return np.float32(loss)
